# revision 35
# baseline (speedup 1.0000x reference)
"""Trainium2 Bass kernel for nn_CollaborativeExpertsWrapper.

Self-contained: shards batch B=128 across 8 NeuronCores (data-parallel
encoders), all-gathers [16, 2048] embeddings, each core redundantly computes
the masked ranking loss; host takes core 0's (loss, acc).

v3: single interleaved emission stream tuned for PE density (HAM stays warm):
 - xT materialized in bf16; all weights f32r full-resident (no wcol re-DMA)
 - o-mean matmuls paced against emitted-PE-work so the in-order PE queue
   never blocks on an o chunk that has not landed
 - pool-before-proj: time-pooling happens on the attention output (DVE reduce
   straight out of PSUM), collapsing the out-projection from 64 to 4 matmuls
   per modality; bo is folded into b2' = bo@W2 + b2
 - queue split: sync HWDGE = o stream + expand weights, scalar HWDGE =
   x tiles + weights + v8 shuffles, gpsimd = collectives only
"""
import sys

sys.path.insert(0, "/opt/trn_rl_repo")

import math
import os
from contextlib import ExitStack

import numpy as np

import concourse.bacc as bacc
import concourse.bass as bass
import concourse.mybir as mybir
import concourse.tile as tile
from concourse.alu_op_type import AluOpType
from concourse.masks import make_identity

F32 = mybir.dt.float32
F32R = mybir.dt.float32r
BF16 = mybir.dt.bfloat16
U8 = mybir.dt.uint8
AF = mybir.ActivationFunctionType
AX = mybir.AxisListType

N_CORES = 8
B = 128
BL = B // N_CORES  # 16 samples per core
T = 64
DIM = 512
HEADS = 4
HD = DIM // HEADS  # 128
MARGIN = 1.0
TOK = BL * T  # 1024 tokens per core per modality
O_T = 1024
ODIM = 512
O_BUFS = 4  # SBUF staging tiles for the o stream (1MB each, half a sample)

_CACHE = {}


def _build():
    nc = bacc.Bacc("TRN2", target_bir_lowering=False, debug=False, num_devices=N_CORES)

    # o and the weights are declared f32r (same bytes as f32) so plain HWDGE
    # loads feed the PE's full-rate f32r path with no cast DMA.
    o_d = nc.dram_tensor("o", [BL, O_T, ODIM], F32R, kind="ExternalInput").ap()
    rgb_d = nc.dram_tensor("rgb", [BL, T, 2048], F32, kind="ExternalInput").ap()
    aud_d = nc.dram_tensor("audio", [BL, T, 128], F32, kind="ExternalInput").ap()
    gm_d = nc.dram_tensor("group_mask", [B], U8, kind="ExternalInput").ap()

    wd = {}
    for m, dm in (("rgb", 2048), ("audio", 128)):
        for p in "qkv":
            wd[f"{m}_W{p}"] = nc.dram_tensor(f"{m}_W{p}", [dm, DIM], F32R, kind="ExternalInput").ap()
            wd[f"{m}_b{p}"] = nc.dram_tensor(f"{m}_b{p}", [DIM], F32R, kind="ExternalInput").ap()
        wd[f"{m}_Wo"] = nc.dram_tensor(f"{m}_Wo", [DIM, DIM], F32R, kind="ExternalInput").ap()
        wd[f"{m}_bo"] = nc.dram_tensor(f"{m}_bo", [DIM], F32R, kind="ExternalInput").ap()
        wd[f"{m}_W2"] = nc.dram_tensor(f"{m}_W2", [DIM, DIM], F32R, kind="ExternalInput").ap()
        wd[f"{m}_b2"] = nc.dram_tensor(f"{m}_b2", [DIM], F32R, kind="ExternalInput").ap()
    wd["expand_W"] = nc.dram_tensor("expand_W", [DIM, 2 * DIM], F32R, kind="ExternalInput").ap()
    wd["expand_b"] = nc.dram_tensor("expand_b", [2 * DIM], F32R, kind="ExternalInput").ap()

    out_d = nc.dram_tensor("out", [1, 2], F32, kind="ExternalOutput").ap()

    stage = os.environ.get("KSTAGE", "full")
    dbg_d = None
    if stage != "full":
        dbg_d = nc.dram_tensor("dbg", [B, 4 * DIM], F32, kind="ExternalOutput").ap()

    with tile.TileContext(nc) as tc:
        _emit(nc, tc, o_d, rgb_d, aud_d, gm_d, wd, out_d, stage, dbg_d)

    nc.compile()
    return nc


def _emit(nc, tc, o_d, rgb_d, aud_d, gm_d, wd, out_d, stage="full", dbg_d=None):
    stk = ExitStack()
    with stk:
        const = stk.enter_context(tc.tile_pool(name="const", bufs=1))
        persist = stk.enter_context(tc.tile_pool(name="persist", bufs=1))
        ps = stk.enter_context(tc.tile_pool(name="psum", bufs=5, space="PSUM"))
        dram = stk.enter_context(tc.tile_pool(name="dram", bufs=1, space="DRAM"))

        def pst(shape, tag="ps", bufs=None):
            return ps.tile(shape, F32, tag=tag, bufs=bufs, name=tag)

        # warmup collective: tiny AllGather issued first on the gpsimd queue
        # (nothing else rides that queue until the real gather), hiding the
        # collective path's fixed setup under the stream phase
        if not os.environ.get("KTIME"):
            warm_in = dram.tile([1, 8], F32)
            warm_out = dram.tile([N_CORES, 8], F32, addr_space="Shared")
            warm_sb = const.tile([1, 8], F32, tag="warm_sb")
            nc.vector.memset(warm_sb[:], 0.0)
            nc.scalar.dma_start(warm_in[:], warm_sb[:])
            nc.gpsimd.collective_compute(
                "AllGather",
                AluOpType.bypass,
                replica_groups=[list(range(N_CORES))],
                ins=[warm_in.opt()],
                outs=[warm_out.opt()],
            )

        # ---------------- constants ----------------
        ident = const.tile([128, 128], F32, tag="ident")
        make_identity(nc, ident)
        ones_col_f32 = const.tile([128, 1], F32, tag="ones_col_f32")
        nc.vector.memset(ones_col_f32[:], 1.0)
        ones64_s = const.tile([128, 128], F32, tag="ones64_s")
        nc.vector.memset(ones64_s[:], 0.0)
        nc.vector.memset(ones64_s[0:64, 0:64], 1.0)
        ones_row_f32 = const.tile([1, 128], F32, tag="ones_row_f32")
        nc.vector.memset(ones_row_f32[:], 1.0)
        ones128 = const.tile([128, 128], F32, tag="ones128")
        nc.vector.memset(ones128[:], 1.0)
        ones_row_r = const.tile([1, 128], F32R, tag="ones_row_r")
        nc.vector.tensor_copy(ones_row_r[:], ones_row_f32[:])
        sel16_s = const.tile([128, BL, BL], F32, tag="sel16_s")
        nc.vector.memset(sel16_s[:], 0.0)
        for b in range(BL):
            nc.vector.memset(sel16_s[:, b, b : b + 1], 1.0)
        sel16 = const.tile([128, BL, BL], F32R, tag="sel16")
        nc.vector.tensor_copy(sel16[:], sel16_s[:])
        # ones64_bf with 128 cols: replicates attention row-sums across all
        # 128 partitions (the av-psum partition range)
        ones64_bf = const.tile([64, 64], BF16, tag="ones64_bf")
        nc.vector.tensor_copy(ones64_bf[:], ones64_s[0:64, 0:64])

        g_row_u8 = const.tile([1, B], U8, tag="g_row_u8")
        nc.scalar.dma_start(g_row_u8[:], gm_d[None, :])
        g_row = const.tile([1, B], F32, tag="g_row")
        nc.vector.tensor_copy(g_row[:], g_row_u8[:])
        g_col_u8 = const.tile([B, 1], U8, tag="g_col_u8")
        nc.scalar.dma_start(g_col_u8[:], gm_d[:, None])
        g_col = const.tile([B, 1], F32, tag="g_col")
        nc.vector.tensor_copy(g_col[:], g_col_u8[:])

        emb_my = persist.tile([BL, 4 * DIM], F32, tag="emb_my")
        feat_sb = emb_my[:, : 2 * DIM]
        oo_sb = emb_my[:, 2 * DIM :]

        # ---------------- o stream ----------------
        # DMA on the sync HWDGE queue; sel-matmuls accumulate into a pinned
        # PSUM bank. pump() emission is paced by emitted-PE-work (see pe_ns)
        # so the in-order PE queue never stalls on an o chunk in flight.
        o_pool = stk.enter_context(tc.tile_pool(name="o_pool", bufs=O_BUFS))
        om_ps = ps.tile([BL, ODIM], F32, tag="ps_om", bufs=1, name="ps_om")
        N_OC = 2 * BL  # 32 chunks of [128, 4, 512] (1MB each)
        o_view = o_d.rearrange("b (h n p) d -> (b h) p n d", p=128, h=2)
        o_state = {"done": 0, "tiles": []}

        # all chunk DMAs are issued upfront on the sync queue: the pool's
        # O_BUFS slots make chunk c's DMA wait (WAR) on the pump-read of
        # chunk c-O_BUFS, so supply self-regulates and is never emission-paced
        for c in range(N_OC):
            t = o_pool.tile([128, O_T // 256, ODIM], F32R, tag="o_tile")
            nc.sync.dma_start(t[:], o_view[c])
            o_state["tiles"].append(t)

        # PE-work accounting: pump o chunks in bursts of BURST chunks per
        # BURST*chunk_ns of emitted PE work -- a burst is a ~3.4us dense
        # matmul run, enough to flip the HAM activity window to full clock
        acct = {"ns": 0.0, "chunk_ns": 5200.0}
        BURST = 4

        def pe_ns(n):
            acct["ns"] += n

        def _pump_one():
            c = o_state["done"]
            t = o_state["tiles"][c]
            b = c // 2
            for j in range(O_T // 256):
                nc.tensor.matmul(
                    om_ps[:],
                    sel16[:, b, :],
                    t[:, j, :],
                    start=(c == 0 and j == 0),
                    stop=(c == N_OC - 1 and j == O_T // 256 - 1),
                )
            o_state["tiles"][c] = None
            o_state["done"] += 1

        def pump(force=False):
            while o_state["done"] < N_OC:
                if not force and acct["ns"] < BURST * acct["chunk_ns"]:
                    return
                for _ in range(BURST):
                    if o_state["done"] >= N_OC:
                        break
                    _pump_one()
                acct["ns"] = max(0.0, acct["ns"] - BURST * acct["chunk_ns"])
                if not force:
                    return

        def o_drain():
            pump(force=True)

        # mask precompute: only needs group_mask, runs early on DVE
        pre_pool = stk.enter_context(tc.tile_pool(name="pre", bufs=1))
        g_pad = pre_pool.tile([B, B], F32, tag="g_pad")
        nc.vector.memset(g_pad[:], 0.0)
        nc.vector.tensor_copy(g_pad[0:1, :], g_row[:])
        gb_ps = ps.tile([B, B], F32, tag="ps_gb", bufs=1, name="ps_gb")
        nc.tensor.matmul(gb_ps[:], ones128[:], g_pad[:], start=True, stop=True)
        gb_sb = pre_pool.tile([B, B], F32, tag="gb_sb")
        nc.vector.tensor_copy(gb_sb[:], gb_ps[:])
        gneg_sb = pre_pool.tile([B, B], F32, tag="gneg_sb")
        nc.vector.tensor_scalar(gneg_sb[:], gb_ps[:], 1e30, -1e30,
                                AluOpType.mult, AluOpType.add)
        pe_ns(300)

        # ---------------- encoders: interleaved emission -----------------
        shared = {}
        rgb_gen = _encoder_gen(nc, tc, pst, const, "rgb", 2048, rgb_d, wd,
                               feat_sb, 0, ident, ones_row_r, ones64_bf,
                               shared, pe_ns)
        aud_gen = _encoder_gen(nc, tc, pst, const, "audio", 128, aud_d, wd,
                               feat_sb, DIM, ident, ones_row_r, ones64_bf,
                               shared, pe_ns)
        # rgb runs alone (with the o pump) until its attention begins; the
        # whole audio encoder is then driven inside rgb's attention window.
        ag_in = dram.tile([BL, 2 * DIM], F32)
        ag_out = dram.tile([B, 2 * DIM], F32, addr_space="Shared")

        def emit_oo():
            # expand weights: loaded late (SBUF freed by encoders) on the
            # right-side stack, behind the last o chunks on the sync queue
            expw_pool = stk.enter_context(
                tc.tile_pool(name="expw", bufs=1, side="right"))
            expw = expw_pool.tile([128, 4, 2 * DIM], F32R, tag="expw")
            nc.sync.dma_start(expw[:], wd["expand_W"].rearrange("(c p) d -> p c d", p=128))
            expb = expw_pool.tile([1, 2 * DIM], F32R, tag="expb")
            nc.sync.dma_start(expb[:], wd["expand_b"][None, :])

            # o-mean -> expand + normalize -> oo
            om_sb = persist.tile([BL, ODIM], F32, tag="om")
            nc.scalar.activation(om_sb[:], om_ps[:], AF.Copy, scale=1.0 / O_T)
            omT = persist.tile([128, 4, BL], F32R, tag="omT")
            for c in range(4):
                tp = pst([128, BL])
                nc.tensor.transpose(tp[:], om_sb[:, 128 * c : 128 * (c + 1)], ident[:BL, :BL])
                nc.scalar.copy(omT[:, c, :], tp[:])
            oo_ps = []
            for half in range(2):
                pp = pst([BL, DIM])
                for c in range(4):
                    nc.tensor.matmul(pp[:], omT[:, c, :],
                                     expw[:, c, 512 * half : 512 * (half + 1)],
                                     start=(c == 0), stop=False)
                nc.tensor.matmul(pp[:], ones_row_r[:, :BL],
                                 expb[:, 512 * half : 512 * (half + 1)],
                                 start=False, stop=True)
                oo_ps.append(pp)
            sq_junk = persist.tile([BL, DIM], F32, tag="sq_junk")
            ss = [persist.tile([BL, 1], F32, tag=f"ss{i}", name=f"ss{i}") for i in range(2)]
            for half in range(2):
                nc.scalar.activation(sq_junk[:], oo_ps[half][:], AF.Square, accum_out=ss[half][:])
            nrm = persist.tile([BL, 1], F32, tag="nrm")
            nc.vector.tensor_tensor(nrm[:], ss[0][:], ss[1][:], AluOpType.add)
            nc.scalar.sqrt(nrm[:], nrm[:])
            nc.vector.tensor_scalar_max(nrm[:], nrm[:], 1e-12)
            rnrm = persist.tile([BL, 1], F32, tag="rnrm")
            with nc.allow_low_precision(reason="oo norm reciprocal; uniform row scale, tolerance 2e-2"):
                nc.vector.reciprocal_approx_fast(rnrm[:], nrm[:])
            for half in range(2):
                nc.vector.tensor_scalar_mul(oo_sb[:, 512 * half : 512 * (half + 1)],
                                            oo_ps[half][:], rnrm[:])

            # early gather: only oo leaves the core; it hides under the tail
            # of the attention phase
            nc.scalar.dma_start(ag_in[:], oo_sb[:])
            if os.environ.get("KTIME"):
                nc.scalar.dma_start(ag_out[0:BL, :], ag_in[:])
            else:
                nc.gpsimd.collective_compute(
                    "AllGather",
                    AluOpType.bypass,
                    replica_groups=[list(range(N_CORES))],
                    ins=[ag_in.opt()],
                    outs=[ag_out.opt()],
                )

        tag = None
        while tag != "attn_start":
            tag = next(rgb_gen)
            if tag == "v":
                # x/weight DMA pressure is over; o can take a bigger share
                acct["chunk_ns"] = 3600.0
            pump()
        next(rgb_gen)  # "w": opens rgb's wo/w2 pool (below the shared pool)
        pump()
        # attention scratch pool is shared between modalities and owned by
        # the driver so pool open/close stays stack-ordered across the
        # interleaved generators
        attn_cm = tc.tile_pool(name="attn_shared", bufs=2)
        shared["attn_pool"] = attn_cm.__enter__()
        # rgb's final pool close is parked on its "end" yield until audio
        # drains (audio shares rgb's attention scratch pool)
        rgb_parked = False
        aud_done = False
        aud_in_attn = False
        oo_emitted = False
        while not (rgb_parked and aud_done):
            if not rgb_parked:
                tag = next(rgb_gen)
                if tag == "end":
                    rgb_parked = True
            if not aud_done:
                try:
                    atag = next(aud_gen)
                    if atag == "attn_start":
                        aud_in_attn = True
                except StopIteration:
                    aud_done = True
            pump()
            if rgb_parked and (aud_in_attn or aud_done) and not oo_emitted:
                # drain the o stream as one dense burst, then compute oo and
                # launch its gather while audio attention is still running
                o_drain()
                emit_oo()
                oo_emitted = True
        attn_cm.__exit__(None, None, None)
        for _ in rgb_gen:
            pass
        if not oo_emitted:
            o_drain()
            emit_oo()

        if stage == "enc":
            nc.sync.dma_start(dbg_d[0:BL, 0 : 2 * DIM], feat_sb[:])
            return
        if stage == "oenc":
            nc.sync.dma_start(dbg_d[0:BL, 0 : 2 * DIM], feat_sb[:])
            nc.sync.dma_start(dbg_d[0:BL, 2 * DIM :], oo_sb[:])
            return

        # ---------------- ranking ----------------
        # each core computes its 16-column Gram slab G[:, mycols] =
        # oo_all @ feat_my^T, transposes it to rows, and a tiny 8KB AllGather
        # assembles the full G^T on every core
        ag2_in = dram.tile([BL, B], F32)
        ag2_out = dram.tile([B, B], F32, addr_space="Shared")
        with tc.tile_pool(name="rank", bufs=1) as rank_pool:
            # featT: [16, 1024] -> [128, 8, 16]
            featT = rank_pool.tile([128, 8, BL], F32R, tag="featT")
            for c2 in range(2):
                tp = pst([128, 4 * BL])
                for j in range(4):
                    ch = 4 * c2 + j
                    nc.tensor.transpose(tp[:, BL * j : BL * (j + 1)],
                                        feat_sb[:, 128 * ch : 128 * (ch + 1)],
                                        ident[:BL, :BL])
                nc.scalar.copy(featT[:, 4 * c2 : 4 * c2 + 4, :],
                               tp[:].rearrange("p (j s) -> p j s", j=4))

            oo_all = rank_pool.tile([B, 2 * DIM], F32, tag="oo_all")
            nc.scalar.dma_start(oo_all[:], ag_out[:])
            ooT = rank_pool.tile([128, 8, B], F32R, tag="ooT")
            for c2 in range(2):
                tp = pst([128, 512])
                for j in range(4):
                    ch = 4 * c2 + j
                    nc.tensor.transpose(tp[:, 128 * j : 128 * (j + 1)],
                                        oo_all[:, 128 * ch : 128 * (ch + 1)], ident[:])
                nc.scalar.copy(ooT[:, 4 * c2 : 4 * c2 + 4, :],
                               tp[:].rearrange("p (j c) -> p j c", j=4))

            slab_ps = pst([B, BL])
            for ch in range(8):
                nc.tensor.matmul(slab_ps[:], ooT[:, ch, :], featT[:, ch, :],
                                 start=(ch == 0), stop=(ch == 7))
            slab_sb = rank_pool.tile([B, BL], F32, tag="slab_sb")
            nc.scalar.copy(slab_sb[:], slab_ps[:])
            slabT_ps = pst([BL, B])
            nc.tensor.transpose(slabT_ps[:], slab_sb[:], ident[:])
            slabT = rank_pool.tile([BL, B], F32, tag="slabT")
            nc.scalar.copy(slabT[:], slabT_ps[:])
            nc.scalar.dma_start(ag2_in[:], slabT[:])
            if os.environ.get("KTIME"):
                nc.scalar.dma_start(ag2_out[0:BL, :], ag2_in[:])
            else:
                nc.gpsimd.collective_compute(
                    "AllGather",
                    AluOpType.bypass,
                    replica_groups=[list(range(N_CORES))],
                    ins=[ag2_in.opt()],
                    outs=[ag2_out.opt()],
                )

            # gathered rows ARE G^T; G follows by one PE transpose, so all
            # cores share bitwise-identical G and G^T
            Gt_sb = rank_pool.tile([B, B], F32, tag="Gt_sb")
            nc.scalar.dma_start(Gt_sb[:], ag2_out[:])
            G_ps = pst([B, B])
            nc.tensor.transpose(G_ps[:], Gt_sb[:], ident[:])
            G_sb = rank_pool.tile([B, B], F32, tag="G_sb")
            nc.scalar.copy(G_sb[:], G_ps[:])

            if stage == "rank1":
                nc.sync.dma_start(dbg_d[:, 0:B], G_sb[:])
                return

            junk = rank_pool.tile([B, B], F32, tag="junk")
            diag = rank_pool.tile([B, 1], F32, tag="diag")
            nc.vector.tensor_tensor(junk[:], G_sb[:], ident[:], AluOpType.mult)
            nc.vector.reduce_sum(diag[:], junk[:], axis=AX.X)
            mdiag = rank_pool.tile([B, 1], F32, tag="mdiag")
            nc.vector.tensor_scalar(mdiag[:], diag[:], -1.0, MARGIN,
                                    AluOpType.mult, AluOpType.add)

            stack = rank_pool.tile([B, 6], F32, tag="stack")

            for di, Gsrc in enumerate((G_sb, Gt_sb)):
                # two independent per-direction chains on separate engines
                eng = nc.vector if di == 0 else nc.gpsimd
                junk_d = rank_pool.tile([B, B], F32, tag=f"junk{di}", name="junk_d")
                Gm = rank_pool.tile([B, B], F32, tag=f"Gm{di}", name="Gm")
                rmax = rank_pool.tile([B, 1], F32, tag=f"rmax{di}", name="rmax")
                top = rank_pool.tile([B, 1], F32, tag=f"top{di}", name="top")
                w = rank_pool.tile([B, 1], F32, tag=f"w{di}", name="w")
                sel = rank_pool.tile([B, 1], F32, tag=f"sel{di}", name="sel")
                eq = rank_pool.tile([B, 1], F32, tag=f"eq{di}", name="eq")
                colv = rank_pool.tile([B, 1], F32, tag=f"colv{di}", name="colv")
                T_sb = rank_pool.tile([B, B], F32, tag=f"T{di}")
                nc.scalar.activation(T_sb[:], Gsrc[:], AF.Relu, bias=mdiag[:])
                eng.tensor_tensor(junk_d[:], T_sb[:], gb_sb[:], AluOpType.mult)
                nc.vector.reduce_sum(w[:], junk_d[:], axis=AX.X)
                nc.vector.tensor_tensor(stack[:, di : di + 1], w[:], g_col[:], AluOpType.mult)
                eng.tensor_tensor(Gm[:], Gsrc[:], gneg_sb[:], AluOpType.add)
                nc.vector.reduce_max(rmax[:], Gm[:], axis=AX.X)
                nc.vector.tensor_tensor(top[:], diag[:], rmax[:], AluOpType.is_ge)
                eng.tensor_tensor(junk_d[:], Gsrc[:], gb_sb[:], AluOpType.mult)
                nc.vector.reduce_sum(sel[:], junk_d[:], axis=AX.X)
                nc.vector.tensor_tensor(sel[:], sel[:], g_col[:], AluOpType.mult)
                nc.vector.tensor_scalar(eq[:], sel[:], 0.0, None, AluOpType.is_equal)
                nc.vector.tensor_scalar(colv[:], eq[:], -1.0, 1.0,
                                        AluOpType.mult, AluOpType.add)
                nc.vector.tensor_copy(stack[:, 4 + di : 5 + di], colv[:])
                nc.vector.tensor_tensor(stack[:, 2 + di : 3 + di], colv[:], top[:],
                                        AluOpType.mult)

            if stage == "rank2":
                nc.sync.dma_start(dbg_d[:, 0:6], stack[:])
                nc.sync.dma_start(dbg_d[:, 8:136], Gt_sb[:])
                return

            S_ps = pst([1, 6])
            nc.tensor.matmul(S_ps[:], ones_col_f32[:], stack[:], start=True, stop=True)
            S_sb = rank_pool.tile([1, 6], F32, tag="S_sb")
            nc.vector.tensor_copy(S_sb[:], S_ps[:])

            sg = rank_pool.tile([1, 1], F32, tag="sg")
            nc.vector.reduce_sum(sg[:], g_row[:], axis=AX.X)

            def sc(tag):
                return rank_pool.tile([1, 1], F32, tag=tag, name=tag)

            t_ls = sc("t_ls")
            nc.vector.tensor_tensor(t_ls[:], S_sb[:, 0:1], S_sb[:, 1:2], AluOpType.add)
            num = sc("num")
            nc.vector.tensor_scalar_mul(num[:], sg[:], -2.0 * MARGIN)
            nc.vector.tensor_tensor(num[:], num[:], t_ls[:], AluOpType.add)
            d1 = sc("d1")
            nc.vector.tensor_scalar(d1[:], sg[:], -1.0, 1.0, AluOpType.add, AluOpType.max)
            ind = sc("ind")
            nc.vector.tensor_scalar(ind[:], sg[:], -1.0, 0.0, AluOpType.add, AluOpType.max)
            nc.vector.tensor_scalar_min(ind[:], ind[:], 1.0)
            nv = sc("nv")
            nc.vector.tensor_tensor(nv[:], ind[:], sg[:], AluOpType.mult)
            d2 = sc("d2")
            nc.vector.tensor_scalar_max(d2[:], nv[:], 1.0)
            r1 = sc("r1")
            nc.vector.reciprocal(r1[:], d1[:])
            r2 = sc("r2")
            nc.vector.reciprocal(r2[:], d2[:])
            out_sb = rank_pool.tile([1, 2], F32, tag="out_sb")
            nc.vector.tensor_tensor(num[:], num[:], r1[:], AluOpType.mult)
            nc.vector.tensor_tensor(out_sb[:, 0:1], num[:], r2[:], AluOpType.mult)

            acc_h = []
            for di in range(2):
                nvx = sc(f"nvx{di}")
                nc.vector.tensor_scalar_max(nvx[:], S_sb[:, 4 + di : 5 + di], 1.0)
                rx = sc(f"rx{di}")
                nc.vector.reciprocal(rx[:], nvx[:])
                ax = sc(f"ax{di}")
                nc.vector.tensor_tensor(ax[:], S_sb[:, 2 + di : 3 + di], rx[:], AluOpType.mult)
                acc_h.append(ax)
            asum = sc("asum")
            nc.vector.tensor_tensor(asum[:], acc_h[0][:], acc_h[1][:], AluOpType.add)
            nc.vector.tensor_scalar_mul(out_sb[:, 1:2], asum[:], 0.5)

            nc.sync.dma_start(out_d[:], out_sb[:])


def _encoder_gen(nc, tc, pst, const, mod, dm, x_d, wd, feat_sb, feat_off,
                 ident, ones_row_r, ones64_bf, shared, pe_ns):
    """Self-attention pooled encoder as a generator: yields at chunk
    boundaries so the driver can interleave the o pump and the other
    modality. All x/weight DMAs go on the scalar HWDGE queue.

    Writes feat_sb[:, feat_off:feat_off+512].
    """
    K = dm // 128
    n_tt = TOK // 128  # 8

    enc_cm = tc.tile_pool(name=f"enc_{mod}", bufs=1)
    enc = enc_cm.__enter__()

    estk = ExitStack()
    xT_pool = estk.enter_context(tc.tile_pool(name=f"xT_{mod}", bufs=1))
    xT = xT_pool.tile([128, K, TOK], F32R, tag="xT")
    flat = x_d.rearrange("b t d -> (b t) d")

    # weight staging: Wv lives on the right-side SBUF stack so it can be
    # freed after v while the left-side qk pool (opened later) persists
    wv_cm = tc.tile_pool(name=f"wv_{mod}", bufs=1, side="right")
    wv_pool = wv_cm.__enter__()
    xload_cm = tc.tile_pool(name=f"xload_{mod}", bufs=2)
    xload = xload_cm.__enter__()
    if dm == 128:
        x_nat_a = xload.tile([128, n_tt, 128], F32, tag="x_nat_a")
        nc.scalar.dma_start(x_nat_a[:], flat.rearrange("(n p) d -> p n d", p=128))
        wv = wv_pool.tile([128, K, DIM], F32R, tag="wv", name="wv")
        bv = wv_pool.tile([1, DIM], F32R, tag="bv")
        nc.scalar.dma_start(wv[:], wd[f"{mod}_Wv"].rearrange("(kc p) d -> p kc d", p=128))
        nc.scalar.dma_start(bv[:], wd[f"{mod}_bv"][None, :])
        for tt in range(n_tt):
            tp = pst([128, 512])
            nc.tensor.transpose(tp[:, :128], x_nat_a[:, tt, :], ident[:])
            nc.scalar.copy(xT[:, 0, 128 * tt : 128 * (tt + 1)], tp[:, :128])
            pe_ns(260)
            if tt % 4 == 3:
                yield "xT"
    else:
        # all x-tile DMAs are queued before the big weight loads so the
        # transpose stream never starves; Wv follows, landing before v starts
        x_nats = [xload.tile([128, dm], F32, tag="x_nat", name="x_nat")]
        nc.scalar.dma_start(x_nats[0][:], flat[0:128, :])
        wv = bv = None
        for tt in range(n_tt):
            if tt + 1 < n_tt:
                nxt = xload.tile([128, dm], F32, tag="x_nat", name="x_nat")
                nc.scalar.dma_start(nxt[:], flat[128 * (tt + 1) : 128 * (tt + 2), :])
                x_nats.append(nxt)
            if tt == n_tt - 1:
                wv = wv_pool.tile([128, K, DIM], F32R, tag="wv", name="wv")
                bv = wv_pool.tile([1, DIM], F32R, tag="bv")
                nc.scalar.dma_start(wv[:], wd[f"{mod}_Wv"].rearrange("(kc p) d -> p kc d", p=128))
                nc.scalar.dma_start(bv[:], wd[f"{mod}_bv"][None, :])
            x_nat = x_nats[tt]
            for kc4 in range(K // 4):
                tp = pst([128, 512])
                for j in range(4):
                    kc = 4 * kc4 + j
                    nc.tensor.transpose(tp[:, 128 * j : 128 * (j + 1)],
                                        x_nat[:, 128 * kc : 128 * (kc + 1)], ident[:])
                nc.scalar.copy(xT[:, 4 * kc4 : 4 * kc4 + 4, 128 * tt : 128 * (tt + 1)],
                               tp[:].rearrange("p (j c) -> p j c", j=4))
                pe_ns(1000)
                yield "xT"
    xload_cm.__exit__(None, None, None)

    # wq is prefetched (left-side slot, reused later by wk) while v computes
    bq_sb = const.tile([128, HEADS], F32R, tag=f"bq_{mod}")
    nc.scalar.dma_start(bq_sb[:], wd[f"{mod}_bq"].rearrange("(o p) -> p o", p=128))
    bk_sb = const.tile([128, HEADS], F32R, tag=f"bk_{mod}")
    nc.scalar.dma_start(bk_sb[:], wd[f"{mod}_bk"].rearrange("(o p) -> p o", p=128))
    wqk_cm = tc.tile_pool(name=f"wqk_{mod}", bufs=1)
    wqk_pool = wqk_cm.__enter__()
    wq = wqk_pool.tile([128, K, DIM], F32R, tag="wqk", name="wq")
    nc.scalar.dma_start(wq[:], wd[f"{mod}_Wq"].rearrange("(kc p) d -> p kc d", p=128))

    # v: lhsT = xT token-tile (stationary), rhs = Wv k-rows (moving)
    v_sb = enc.tile([128, n_tt, DIM], BF16, tag="v_sb")
    for tt in range(n_tt):
        pv = pst([128, DIM])
        for kc in range(K):
            nc.tensor.matmul(pv[:], xT[:, kc, 128 * tt : 128 * (tt + 1)], wv[:, kc, :],
                             start=(kc == 0), stop=False)
        nc.tensor.matmul(pv[:], ones_row_r[:], bv[:], start=False, stop=True)
        nc.vector.tensor_copy(v_sb[:, tt, :], pv[:])
        pe_ns(230 * (K + 1))
        yield "v"
    wv_cm.__exit__(None, None, None)  # right-side stack: frees independently

    # wk loads into Wv's freed right-side space, overlapping q's compute
    wk_cm = tc.tile_pool(name=f"wk_{mod}", bufs=1, side="right")
    wk_pool = wk_cm.__enter__()
    wk = wk_pool.tile([128, K, DIM], F32R, tag="wk", name="wk")
    nc.scalar.dma_start(wk[:], wd[f"{mod}_Wk"].rearrange("(kc p) d -> p kc d", p=128))

    # q, k: lhsT = W column-block (stationary), rhs = xT (moving) -> [d, tok]
    qT = enc.tile([128, HEADS, TOK], BF16, tag="qT")
    kT = enc.tile([128, HEADS, TOK], BF16, tag="kT")
    for wfull, outT, b_sb in ((wq, qT, bq_sb), (wk, kT, bk_sb)):
        for dt_ in range(HEADS):
            for blk in range(TOK // 512):
                pq = pst([128, 512])
                for kc in range(K):
                    nc.tensor.matmul(pq[:], wfull[:, kc, 128 * dt_ : 128 * (dt_ + 1)],
                                     xT[:, kc, 512 * blk : 512 * (blk + 1)],
                                     start=(kc == 0), stop=(kc == K - 1))
                nc.scalar.activation(outT[:, dt_, 512 * blk : 512 * (blk + 1)], pq[:],
                                     AF.Identity, bias=b_sb[:, dt_ : dt_ + 1])
                pe_ns(230 * K)
                yield "qk"
    wk_cm.__exit__(None, None, None)
    wqk_cm.__exit__(None, None, None)
    estk.close()  # frees xT
    yield "attn_start"

    # attention, grp-outer: reshuffle 8 samples of v to base partition 0 via
    # SBUF->SBUF DMA (engines cannot shift partitions), then per-head flow.
    # Time-pooling happens here: av psum is DVE-reduced straight into pooled.
    scale = 1.0 / math.sqrt(HD)
    # pooled attention output (time-summed), per head: [hd, h, samples]
    pooled = enc.tile([128, HEADS, BL], F32R, tag="pooled")
    lstk = ExitStack()
    wo_pool = lstk.enter_context(tc.tile_pool(name=f"wo_{mod}", bufs=1))
    wo = wo_pool.tile([128, HEADS, DIM], F32R, tag="wo")
    nc.scalar.dma_start(wo[:], wd[f"{mod}_Wo"].rearrange("(h p) d -> p h d", p=128))
    w2 = wo_pool.tile([128, HEADS, DIM], F32R, tag="w2")
    nc.scalar.dma_start(w2[:], wd[f"{mod}_W2"].rearrange("(c p) d -> p c d", p=128))
    b2 = wo_pool.tile([1, DIM], F32R, tag="b2")
    nc.scalar.dma_start(b2[:], wd[f"{mod}_b2"][None, :])
    boT = wo_pool.tile([128, HEADS], F32R, tag="boT")
    nc.scalar.dma_start(boT[:], wd[f"{mod}_bo"].rearrange("(o p) -> p o", p=128))
    # b2' = bo @ W2 + b2 (folds the out-proj bias through W2)
    b2p_ps = pst([1, DIM])
    for c in range(HEADS):
        nc.tensor.matmul(b2p_ps[:], boT[:, c : c + 1], w2[:, c, :],
                         start=(c == 0), stop=False)
    nc.tensor.matmul(b2p_ps[:], ones_row_r[:, 0:1], b2[:], start=False, stop=True)
    b2p = wo_pool.tile([1, DIM], F32R, tag="b2p")
    nc.scalar.copy(b2p[:], b2p_ps[:])
    pe_ns(300)
    yield "w"

    ap = shared["attn_pool"]  # driver-owned shared scratch
    for grp in range(BL // 8):
        v8 = ap.tile([64, 8, DIM], BF16, tag="v8", bufs=2)
        v8v = v8[:].rearrange("p (ul half) d -> p ul half d", half=2)
        nc.scalar.dma_start(v8v[:, :, 0, :], v_sb[0:64, 4 * grp : 4 * grp + 4, :])
        nc.scalar.dma_start(v8v[:, :, 1, :], v_sb[64:128, 4 * grp : 4 * grp + 4, :])
        for h in range(HEADS):
            sT8 = pst([64, 512])
            for i in range(8):
                b = 8 * grp + i
                nc.tensor.matmul(sT8[:, 64 * i : 64 * (i + 1)],
                                 kT[:, h, 64 * b : 64 * (b + 1)],
                                 qT[:, h, 64 * b : 64 * (b + 1)],
                                 start=True, stop=True)
            exps = ap.tile([64, 512], BF16, tag="exps")
            nc.scalar.activation(exps[:], sT8[:], AF.Exp, scale=scale)
            pe_ns(500)
            yield "attn_s"
            rs = pst([64, 512])
            nc.tensor.matmul(rs[:], ones64_bf[:], exps[:], start=True, stop=True)
            rrs = ap.tile([64, 512], F32, tag="rrs")
            nc.vector.reciprocal_approx_fast(rrs[:], rs[:])
            aT8 = ap.tile([64, 512], BF16, tag="aT8")
            nc.vector.tensor_tensor(aT8[:], exps[:], rrs[:], AluOpType.mult)
            pe_ns(250)
            yield "attn_m"
            avp = pst([128, 512])
            for i in range(8):
                nc.tensor.matmul(avp[:, 64 * i : 64 * (i + 1)],
                                 v8[:, i, 128 * h : 128 * (h + 1)],
                                 aT8[:, 64 * i : 64 * (i + 1)],
                                 start=True, stop=True)
            # time-pool the attention output for these 8 samples directly
            # from psum: [128, (8s 64t)] -> [128, 8]
            with nc.allow_low_precision(reason="f32r is bit-identical to f32; accumulation is full fp32"):
                nc.vector.reduce_sum(pooled[:, h, 8 * grp : 8 * grp + 8],
                                     avp[:].rearrange("p (s t) -> p s t", t=T),
                                     axis=AX.X)
            pe_ns(600)
            yield "attn_v"

    # out-proj on pooled sums: out1 = (pooled/T) @ Wo  (bo folded into b2')
    out1_ps = pst([BL, DIM])
    for h in range(HEADS):
        nc.tensor.matmul(out1_ps[:], pooled[:, h, :], wo[:, h, :],
                         start=(h == 0), stop=(h == HEADS - 1))
    out1 = wo_pool.tile([BL, DIM], F32, tag="out1")
    nc.scalar.activation(out1[:], out1_ps[:], AF.Copy, scale=1.0 / T)
    pe_ns(900)
    yield "proj"

    out1T = wo_pool.tile([128, HEADS, BL], F32R, tag="out1T")
    for c in range(HEADS):
        tp = pst([128, BL])
        nc.tensor.transpose(tp[:], out1[:, 128 * c : 128 * (c + 1)], ident[:BL, :BL])
        nc.scalar.copy(out1T[:, c, :], tp[:])
    pf = pst([BL, DIM])
    for c in range(HEADS):
        nc.tensor.matmul(pf[:], out1T[:, c, :], w2[:, c, :], start=(c == 0), stop=False)
    nc.tensor.matmul(pf[:], ones_row_r[:, :BL], b2p[:], start=False, stop=True)
    nc.scalar.copy(feat_sb[:, feat_off : feat_off + DIM], pf[:])
    pe_ns(1500)
    yield "end"
    lstk.close()
    enc_cm.__exit__(None, None, None)


def kernel(**inputs):
    if "runner" not in _CACHE:
        _CACHE["runner"] = _make_runner()
    return _CACHE["runner"](inputs)


def _make_runner():
    nc = _build()
    import jax
    from jax.sharding import Mesh, PartitionSpec
    from jax.experimental.shard_map import shard_map
    from concourse import bass2jax

    bass2jax.install_neuronx_cc_hook()

    partition_name = nc.partition_id_tensor.name if nc.partition_id_tensor else None
    in_names, out_names, out_avals, zero_outs = [], [], [], []
    for alloc in nc.m.functions[0].allocations:
        if not isinstance(alloc, mybir.MemoryLocationSet):
            continue
        name = alloc.memorylocations[0].name
        if alloc.kind == "ExternalInput":
            if name != partition_name:
                in_names.append(name)
        elif alloc.kind == "ExternalOutput":
            out_names.append(name)
            shape = tuple(alloc.tensor_shape)
            dtype = mybir.dt.np(alloc.dtype)
            out_avals.append(jax.core.ShapedArray(shape, dtype))
            zero_outs.append(np.zeros(shape, dtype))
    n_params = len(in_names)
    all_in_names = list(in_names) + list(out_names)
    if partition_name is not None:
        all_in_names.append(partition_name)

    def _body(*args):
        operands = list(args)
        if partition_name is not None:
            operands.append(bass2jax.partition_id_tensor())
        outs = bass2jax._bass_exec_p.bind(
            *operands,
            out_avals=tuple(out_avals),
            in_names=tuple(all_in_names),
            out_names=tuple(out_names),
            lowering_input_output_aliases=(),
            sim_require_finite=True,
            sim_require_nnan=True,
            nc=nc,
        )
        return tuple(outs)

    devices = jax.devices()[:N_CORES]
    mesh = Mesh(np.asarray(devices), ("core",))
    in_specs = (PartitionSpec("core"),) * (n_params + len(out_names))
    out_specs = (PartitionSpec("core"),) * len(out_names)
    sharded = jax.jit(
        shard_map(_body, mesh=mesh, in_specs=in_specs, out_specs=out_specs,
                  check_rep=False),
        keep_unused=True,
    )

    out_idx = out_names.index("out")

    def run(inputs):
        per_core = _shard_inputs(inputs)
        concat_in = [
            np.concatenate([per_core[c][name] for c in range(N_CORES)], axis=0)
            for name in in_names
        ]
        concat_zeros = [
            np.zeros((N_CORES * z.shape[0], *z.shape[1:]), z.dtype) for z in zero_outs
        ]
        out_arrs = sharded(*concat_in, *concat_zeros)
        run.last_outputs = {n: np.asarray(out_arrs[i]) for i, n in enumerate(out_names)}
        out = run.last_outputs["out"]  # [8, 2]
        return np.float32(out[0, 0]), np.float32(out[0, 1])

    run.sharded = sharded
    run.in_names = in_names
    run.zero_outs = zero_outs
    run.nc = nc
    return run


def _shard_inputs(inputs):
    per_core = []
    gm = np.ascontiguousarray(np.asarray(inputs["group_mask"]).astype(np.uint8))
    shared = {}
    for k, v in inputs.items():
        if k not in ("o", "rgb", "audio", "group_mask"):
            shared[k] = np.ascontiguousarray(np.asarray(v, dtype=np.float32))
    o = np.asarray(inputs["o"], dtype=np.float32)
    rgb = np.asarray(inputs["rgb"], dtype=np.float32)
    audio = np.asarray(inputs["audio"], dtype=np.float32)
    for c in range(N_CORES):
        sl = slice(BL * c, BL * (c + 1))
        m = {
            "o": np.ascontiguousarray(o[sl]),
            "rgb": np.ascontiguousarray(rgb[sl]),
            "audio": np.ascontiguousarray(audio[sl]),
            "group_mask": gm,
        }
        m.update(shared)
        per_core.append(m)
    return per_core


# revision 103
# speedup vs baseline: 1.0910x; 1.0910x over previous
"""Trainium2 Bass kernel for nn_CollaborativeExpertsWrapper.

Self-contained: shards batch B=128 across 8 NeuronCores (data-parallel
encoders), all-gathers [16, 2048] embeddings, each core redundantly computes
the masked ranking loss; host takes core 0's (loss, acc).

v3: single interleaved emission stream tuned for PE density (HAM stays warm):
 - xT materialized in bf16; all weights f32r full-resident (no wcol re-DMA)
 - o-mean matmuls paced against emitted-PE-work so the in-order PE queue
   never blocks on an o chunk that has not landed
 - pool-before-proj: time-pooling happens on the attention output (DVE reduce
   straight out of PSUM), collapsing the out-projection from 64 to 4 matmuls
   per modality; bo is folded into b2' = bo@W2 + b2
 - queue split: sync HWDGE = o stream + expand weights, scalar HWDGE =
   x tiles + weights + v8 shuffles, gpsimd = collectives only
"""
import sys

sys.path.insert(0, "/opt/trn_rl_repo")

import math
import os
from contextlib import ExitStack

import numpy as np

import concourse.bacc as bacc
import concourse.bass as bass
import concourse.mybir as mybir
import concourse.tile as tile
from concourse.alu_op_type import AluOpType
from concourse.masks import make_identity

F32 = mybir.dt.float32
F32R = mybir.dt.float32r
BF16 = mybir.dt.bfloat16
U8 = mybir.dt.uint8
AF = mybir.ActivationFunctionType
AX = mybir.AxisListType

N_CORES = 8
B = 128
BL = B // N_CORES  # 16 samples per core
T = 64
DIM = 512
HEADS = 4
HD = DIM // HEADS  # 128
MARGIN = 1.0
TOK = BL * T  # 1024 tokens per core per modality
O_T = 1024
ODIM = 512
O_BUFS = 4  # SBUF staging tiles for the o stream (1MB each, half a sample)

_CACHE = {}


def _build():
    nc = bacc.Bacc("TRN2", target_bir_lowering=False, debug=False, num_devices=N_CORES)

    # qkv weights are declared f32 (cast to bf16 during the SWDGE load);
    # o is f32 (accumulate-DMA) and bitcast to f32r for the drain matmuls;
    # the remaining weights are f32r for the PE's full-rate path.
    o_d = nc.dram_tensor("o", [BL, O_T, ODIM], F32R, kind="ExternalInput").ap()
    rgb_d = nc.dram_tensor("rgb", [BL, T, 2048], F32, kind="ExternalInput").ap()
    aud_d = nc.dram_tensor("audio", [BL, T, 128], F32, kind="ExternalInput").ap()
    gm_d = nc.dram_tensor("group_mask", [B], U8, kind="ExternalInput").ap()

    wd = {}
    for m, dm in (("rgb", 2048), ("audio", 128)):
        for p in "qkv":
            wd[f"{m}_W{p}"] = nc.dram_tensor(f"{m}_W{p}", [dm, DIM], F32R, kind="ExternalInput").ap()
            wd[f"{m}_b{p}"] = nc.dram_tensor(f"{m}_b{p}", [DIM], F32R, kind="ExternalInput").ap()
        wd[f"{m}_Wo"] = nc.dram_tensor(f"{m}_Wo", [DIM, DIM], F32R, kind="ExternalInput").ap()
        wd[f"{m}_bo"] = nc.dram_tensor(f"{m}_bo", [DIM], F32R, kind="ExternalInput").ap()
        wd[f"{m}_W2"] = nc.dram_tensor(f"{m}_W2", [DIM, DIM], F32R, kind="ExternalInput").ap()
        wd[f"{m}_b2"] = nc.dram_tensor(f"{m}_b2", [DIM], F32R, kind="ExternalInput").ap()
    wd["expand_W"] = nc.dram_tensor("expand_W", [DIM, 2 * DIM], F32R, kind="ExternalInput").ap()
    wd["expand_b"] = nc.dram_tensor("expand_b", [2 * DIM], F32R, kind="ExternalInput").ap()

    out_d = nc.dram_tensor("out", [1, 2], F32, kind="ExternalOutput").ap()

    stage = os.environ.get("KSTAGE", "full")
    dbg_d = None
    if stage != "full":
        dbg_d = nc.dram_tensor("dbg", [B, 4 * DIM], F32, kind="ExternalOutput").ap()

    with tile.TileContext(nc) as tc:
        _emit(nc, tc, o_d, rgb_d, aud_d, gm_d, wd, out_d, stage, dbg_d)

    nc.compile()
    return nc


def _emit(nc, tc, o_d, rgb_d, aud_d, gm_d, wd, out_d, stage="full", dbg_d=None):
    stk = ExitStack()
    with stk:
        const = stk.enter_context(tc.tile_pool(name="const", bufs=1))
        persist = stk.enter_context(tc.tile_pool(name="persist", bufs=1))
        ps = stk.enter_context(tc.tile_pool(name="psum", bufs=3, space="PSUM"))
        dram = stk.enter_context(tc.tile_pool(name="dram", bufs=1, space="DRAM"))

        def pst(shape, tag="ps", bufs=None):
            return ps.tile(shape, F32, tag=tag, bufs=bufs, name=tag)

        # warmup collective: tiny AllGather issued first on the gpsimd queue
        # (nothing else rides that queue until the real gather), hiding the
        # collective path's fixed setup under the stream phase
        if not os.environ.get("KTIME"):
            warm_in = dram.tile([1, 8], F32)
            warm_out = dram.tile([N_CORES, 8], F32, addr_space="Shared")
            warm_sb = const.tile([1, 8], F32, tag="warm_sb")
            nc.vector.memset(warm_sb[:], 0.0)
            nc.scalar.dma_start(warm_in[:], warm_sb[:])
            nc.gpsimd.collective_compute(
                "AllGather",
                AluOpType.bypass,
                replica_groups=[list(range(N_CORES))],
                ins=[warm_in.opt()],
                outs=[warm_out.opt()],
            )

        # ---------------- constants ----------------
        ident = const.tile([128, 128], F32, tag="ident")
        make_identity(nc, ident)
        ones_col_f32 = const.tile([128, 1], F32, tag="ones_col_f32")
        nc.vector.memset(ones_col_f32[:], 1.0)
        ones64_s = const.tile([128, 128], F32, tag="ones64_s")
        nc.vector.memset(ones64_s[:], 0.0)
        nc.vector.memset(ones64_s[0:64, 0:64], 1.0)
        ones_row_f32 = const.tile([1, 128], F32, tag="ones_row_f32")
        nc.vector.memset(ones_row_f32[:], 1.0)
        ones128 = const.tile([128, 128], F32, tag="ones128")
        nc.vector.memset(ones128[:], 1.0)
        ones_row_r = const.tile([1, 128], F32R, tag="ones_row_r")
        nc.vector.tensor_copy(ones_row_r[:], ones_row_f32[:])
        ones_col_r = const.tile([128, 1], F32R, tag="ones_col_r")
        nc.vector.tensor_copy(ones_col_r[:], ones_col_f32[:])
        sel16_s = const.tile([128, BL, BL], F32, tag="sel16_s")
        nc.vector.memset(sel16_s[:], 0.0)
        for b in range(BL):
            nc.vector.memset(sel16_s[:, b, b : b + 1], 1.0)
        sel16 = const.tile([128, BL, BL], F32R, tag="sel16")
        nc.vector.tensor_copy(sel16[:], sel16_s[:])
        # ones64_bf with 128 cols: replicates attention row-sums across all
        # 128 partitions (the av-psum partition range)
        ones64_bf = const.tile([64, 64], BF16, tag="ones64_bf")
        nc.vector.tensor_copy(ones64_bf[:], ones64_s[0:64, 0:64])

        g_row_u8 = const.tile([1, B], U8, tag="g_row_u8")
        nc.scalar.dma_start(g_row_u8[:], gm_d[None, :])
        g_row = const.tile([1, B], F32, tag="g_row")
        nc.vector.tensor_copy(g_row[:], g_row_u8[:])
        g_col_u8 = const.tile([B, 1], U8, tag="g_col_u8")
        nc.scalar.dma_start(g_col_u8[:], gm_d[:, None])
        g_col = const.tile([B, 1], F32, tag="g_col")
        nc.vector.tensor_copy(g_col[:], g_col_u8[:])

        emb_my = persist.tile([BL, 4 * DIM], F32, tag="emb_my")
        feat_sb = emb_my[:, : 2 * DIM]
        oo_sb = emb_my[:, 2 * DIM :]
        out1_scr = persist.tile([BL, DIM], F32, tag="out1_scr")

        # ---------------- weights ----------------
        # rgb qkv weights: plain f32r HWDGE loads into a rotating right-side
        # staging slot, cast to resident bf16 by the otherwise-idle gpsimd
        # engine. bf16 stationaries get pipelined (fast) weight loads on the
        # PE, unlike f32r whose 4-byte load is fused into every matmul.
        # audio's weights stay f32r (too few matmuls to matter).
        wt = {}
        rgbw_cm = tc.tile_pool(name="rgbw", bufs=1, side="right")
        rgbw = rgbw_cm.__enter__()
        for p in "vqk":
            wt[f"rgb_{p}"] = rgbw.tile([128, 16, DIM], BF16, tag=f"w{p}_rgb",
                                       name=f"w{p}_rgb")
        stage_cm = tc.tile_pool(name="wstage", bufs=1, side="right")
        stage_pool = stage_cm.__enter__()

        def emit_wstage():
            # weight stages ride the sync queue AHEAD of the o chunks (x rides
            # scalar); quarter-size double-buffered slots pipeline the DVE
            # casts against the next load
            for p in "vqk":
                w_src = wd[f"rgb_W{p}"].rearrange("(kc p) d -> p kc d", p=128)
                for h in range(2):
                    st = stage_pool.tile([128, 8, DIM], F32R, tag="wstage",
                                         name=f"st_{p}{h}")
                    nc.scalar.dma_start(st[:], w_src[:, 8 * h : 8 * (h + 1), :])
                    for kc in range(8):
                        nc.vector.tensor_copy(wt[f"rgb_{p}"][:, 8 * h + kc, :],
                                              st[:, kc, :])
            stage_cm.__exit__(None, None, None)

        for p in "vqk":
            t = persist.tile([128, 1, DIM], F32R, tag=f"w{p}_aud", name=f"w{p}_aud")
            nc.scalar.dma_start(t[:], wd[f"audio_W{p}"].rearrange("(kc p) d -> p kc d", p=128))
            wt[f"audio_{p}"] = t

        # ---------------- o stream ----------------
        # chunk DMAs ride the sync HWDGE queue, all issued upfront: the
        # pool's O_BUFS slots make chunk c's DMA wait (WAR) on the pump-read
        # of chunk c-O_BUFS, so supply self-regulates. pump() emits bursts of
        # BURST dense chunks (a ~3.4us matmul run flips the HAM activity
        # window to full clock), paced by per-yield PE-work estimates so the
        # in-order PE queue never stalls on a chunk in flight.
        o_pool = stk.enter_context(tc.tile_pool(name="o_pool", bufs=O_BUFS))
        om_ps = ps.tile([BL, ODIM], F32, tag="ps_om", bufs=1, name="ps_om")
        N_OC = 2 * BL  # 32 chunks of [128, 4, 512] (1MB each)
        o_view = o_d.rearrange("b (h n p) d -> (b h) p n d", p=128, h=2)
        o_state = {"issued": 0, "done": 0, "tiles": [None] * N_OC}

        def o_issue():
            c = o_state["issued"]
            if c >= N_OC:
                return
            t = o_pool.tile([128, O_T // 256, ODIM], F32R, tag="o_tile")
            nc.sync.dma_start(t[:], o_view[c])
            o_state["tiles"][c] = t
            o_state["issued"] += 1

        acct = {"ns": 0.0, "chunk_ns": 5200.0, "burst": 4}

        def pump(force=False):
            while o_state["done"] < N_OC:
                burst = acct["burst"]
                if not force and acct["ns"] < burst * acct["chunk_ns"]:
                    return
                for _ in range(burst):
                    c = o_state["done"]
                    if c >= N_OC:
                        break
                    while o_state["issued"] < min(c + O_BUFS, N_OC):
                        o_issue()
                    t = o_state["tiles"][c]
                    b = c // 2
                    for j in range(O_T // 256):
                        nc.tensor.matmul(
                            om_ps[:], sel16[:, b, :], t[:, j, :],
                            start=(c == 0 and j == 0),
                            stop=(c == N_OC - 1 and j == O_T // 256 - 1))
                    o_state["tiles"][c] = None
                    o_state["done"] += 1
                acct["ns"] = max(0.0, acct["ns"] - burst * acct["chunk_ns"])
                if not force:
                    return

        # small upfront cushion; the rest issues on consumption, naturally
        # interleaving with the generator x loads on the sync queue
        for _ in range(2):
            o_issue()

        def o_drain():
            pump(force=True)

        # mask precompute: only needs group_mask, runs early on DVE
        pre_pool = stk.enter_context(tc.tile_pool(name="pre", bufs=1))
        g_pad = pre_pool.tile([B, B], F32, tag="g_pad")
        nc.vector.memset(g_pad[:], 0.0)
        nc.vector.tensor_copy(g_pad[0:1, :], g_row[:])
        gb_ps = ps.tile([B, B], F32, tag="ps_gb", bufs=1, name="ps_gb")
        nc.tensor.matmul(gb_ps[:], ones128[:], g_pad[:], start=True, stop=True)
        gb_sb = pre_pool.tile([B, B], F32, tag="gb_sb")
        nc.vector.tensor_copy(gb_sb[:], gb_ps[:])
        gneg_sb = pre_pool.tile([B, B], F32, tag="gneg_sb")
        nc.vector.tensor_scalar(gneg_sb[:], gb_ps[:], 1e30, -1e30,
                                AluOpType.mult, AluOpType.add)
        pe_ns(300)

        # ---------------- encoders: interleaved emission -----------------
        shared = {"out1_scr": out1_scr}
        rgb_gen = _encoder_gen(nc, tc, pst, const, "rgb", 2048, rgb_d, wd, wt,
                               feat_sb, 0, ident, ones_row_r, ones64_bf, shared)
        aud_gen = _encoder_gen(nc, tc, pst, const, "audio", 128, aud_d, wd, wt,
                               feat_sb, DIM, ident, ones_row_r, ones64_bf, shared)
        # rgb runs alone (with the o pump) until its attention begins; the
        # whole audio encoder is then driven inside rgb's attention window.
        ag_in = dram.tile([BL, 2 * DIM], F32)
        ag_out = dram.tile([B, 2 * DIM], F32, addr_space="Shared")

        def emit_oo():
            # expand weights: loaded late (SBUF freed by encoders) on the
            # right-side stack, behind the last o chunks on the sync queue
            expw_pool = stk.enter_context(
                tc.tile_pool(name="expw", bufs=1, side="right"))
            expw = expw_pool.tile([128, 4, 2 * DIM], F32R, tag="expw")
            nc.sync.dma_start(expw[:], wd["expand_W"].rearrange("(c p) d -> p c d", p=128))
            expb = expw_pool.tile([1, 2 * DIM], F32R, tag="expb")
            nc.sync.dma_start(expb[:], wd["expand_b"][None, :])

            # o-mean -> expand + normalize -> oo
            om_sb = persist.tile([BL, ODIM], F32, tag="om")
            nc.scalar.activation(om_sb[:], om_ps[:], AF.Copy, scale=1.0 / O_T)
            omT = persist.tile([128, 4, BL], F32R, tag="omT")
            for c in range(4):
                tp = pst([128, BL])
                nc.tensor.transpose(tp[:], om_sb[:, 128 * c : 128 * (c + 1)], ident[:BL, :BL])
                nc.scalar.copy(omT[:, c, :], tp[:])
            oo_ps = []
            for half in range(2):
                pp = pst([BL, DIM])
                for c in range(4):
                    nc.tensor.matmul(pp[:], omT[:, c, :],
                                     expw[:, c, 512 * half : 512 * (half + 1)],
                                     start=(c == 0), stop=False)
                nc.tensor.matmul(pp[:], ones_row_r[:, :BL],
                                 expb[:, 512 * half : 512 * (half + 1)],
                                 start=False, stop=True)
                oo_ps.append(pp)
            sq_junk = persist.tile([BL, DIM], F32, tag="sq_junk")
            ss = [persist.tile([BL, 1], F32, tag=f"ss{i}", name=f"ss{i}") for i in range(2)]
            for half in range(2):
                nc.scalar.activation(sq_junk[:], oo_ps[half][:], AF.Square, accum_out=ss[half][:])
            nrm = persist.tile([BL, 1], F32, tag="nrm")
            nc.vector.tensor_tensor(nrm[:], ss[0][:], ss[1][:], AluOpType.add)
            nc.scalar.sqrt(nrm[:], nrm[:])
            nc.vector.tensor_scalar_max(nrm[:], nrm[:], 1e-12)
            rnrm = persist.tile([BL, 1], F32, tag="rnrm")
            with nc.allow_low_precision(reason="oo norm reciprocal; uniform row scale, tolerance 2e-2"):
                nc.vector.reciprocal_approx_fast(rnrm[:], nrm[:])
            for half in range(2):
                nc.vector.tensor_scalar_mul(oo_sb[:, 512 * half : 512 * (half + 1)],
                                            oo_ps[half][:], rnrm[:])

            # early gather: only oo leaves the core; it hides under the tail
            # of the attention phase
            nc.scalar.dma_start(ag_in[:], oo_sb[:])
            if os.environ.get("KTIME"):
                nc.scalar.dma_start(ag_out[0:BL, :], ag_in[:])
            else:
                nc.gpsimd.collective_compute(
                    "AllGather",
                    AluOpType.bypass,
                    replica_groups=[list(range(N_CORES))],
                    ins=[ag_in.opt()],
                    outs=[ag_out.opt()],
                )

        # per-yield PE-work estimates drive the pump pacing
        # attention stages are wall-clock-bound, not PE-bound: their COST is
        # deliberately inflated so the pump keeps feeding o chunks into the
        # dependency gaps instead of leaving a bulk drain for the end
        COST = {"xT": 1000, "v": 2600, "qk": 2500, "attn_s": 1500,
                "attn_m": 800, "attn_v": 1500, "w": 300, "proj": 1500,
                "end": 1500}
        tag = next(rgb_gen)  # first x tiles queued on sync
        acct["ns"] += COST.get(tag, 300)
        emit_wstage()  # big weight loads go behind the small ones on scalar
        while tag != "attn_start":
            tag = next(rgb_gen)
            acct["ns"] += COST.get(tag, 300)
            if tag == "v":
                # x/weight DMA pressure is easing; o can take a bigger share
                acct["chunk_ns"] = 3600.0
            pump()
        rgbw_cm.__exit__(None, None, None)  # rgb qkv weights done
        next(rgb_gen)  # "w": opens rgb's wo/w2 pool (below the shared pool)
        pump()
        # attention scratch pool is shared between modalities and owned by
        # the driver so pool open/close stays stack-ordered across the
        # interleaved generators
        attn_cm = tc.tile_pool(name="attn_shared", bufs=2)
        shared["attn_pool"] = attn_cm.__enter__()
        # rgb's final pool close is parked on its "end" yield until audio
        # drains (audio shares rgb's attention scratch pool)
        rgb_parked = False
        aud_done = False
        aud_in_attn = False
        oo_emitted = False
        while not (rgb_parked and aud_done):
            if not rgb_parked:
                tag = next(rgb_gen)
                acct["ns"] += COST.get(tag, 300)
                if tag == "end":
                    rgb_parked = True
            if not aud_done:
                try:
                    atag = next(aud_gen)
                    acct["ns"] += COST.get(atag, 300)
                    if atag == "attn_start":
                        aud_in_attn = True
                except StopIteration:
                    aud_done = True
            pump()
            if rgb_parked and (aud_in_attn or aud_done) and not oo_emitted:
                # drain the o stream as one dense burst, then compute oo and
                # launch its gather while audio attention is still running
                o_drain()
                emit_oo()
                oo_emitted = True
        attn_cm.__exit__(None, None, None)
        for _ in rgb_gen:
            pass
        if not oo_emitted:
            o_drain()
            emit_oo()

        if stage == "enc":
            nc.sync.dma_start(dbg_d[0:BL, 0 : 2 * DIM], feat_sb[:])
            return
        if stage == "oenc":
            nc.sync.dma_start(dbg_d[0:BL, 0 : 2 * DIM], feat_sb[:])
            nc.sync.dma_start(dbg_d[0:BL, 2 * DIM :], oo_sb[:])
            return

        # ---------------- ranking ----------------
        # each core computes its 16-column Gram slab G[:, mycols] =
        # oo_all @ feat_my^T, transposes it to rows, and a tiny 8KB AllGather
        # assembles the full G^T on every core
        ag2_in = dram.tile([BL, B], F32)
        ag2_out = dram.tile([B, B], F32, addr_space="Shared")
        with tc.tile_pool(name="rank", bufs=1) as rank_pool:
            # featT: [16, 1024] -> [128, 8, 16]
            featT = rank_pool.tile([128, 8, BL], F32R, tag="featT")
            for c2 in range(2):
                tp = pst([128, 4 * BL])
                for j in range(4):
                    ch = 4 * c2 + j
                    nc.tensor.transpose(tp[:, BL * j : BL * (j + 1)],
                                        feat_sb[:, 128 * ch : 128 * (ch + 1)],
                                        ident[:BL, :BL])
                nc.scalar.copy(featT[:, 4 * c2 : 4 * c2 + 4, :],
                               tp[:].rearrange("p (j s) -> p j s", j=4))

            oo_all = rank_pool.tile([B, 2 * DIM], F32, tag="oo_all")
            nc.scalar.dma_start(oo_all[:], ag_out[:])
            ooT = rank_pool.tile([128, 8, B], F32R, tag="ooT")
            for c2 in range(2):
                tp = pst([128, 512])
                for j in range(4):
                    ch = 4 * c2 + j
                    nc.tensor.transpose(tp[:, 128 * j : 128 * (j + 1)],
                                        oo_all[:, 128 * ch : 128 * (ch + 1)], ident[:])
                nc.scalar.copy(ooT[:, 4 * c2 : 4 * c2 + 4, :],
                               tp[:].rearrange("p (j c) -> p j c", j=4))

            slab_ps = pst([B, BL])
            for ch in range(8):
                nc.tensor.matmul(slab_ps[:], ooT[:, ch, :], featT[:, ch, :],
                                 start=(ch == 0), stop=(ch == 7))
            slab_sb = rank_pool.tile([B, BL], F32, tag="slab_sb")
            nc.scalar.copy(slab_sb[:], slab_ps[:])
            slabT_ps = pst([BL, B])
            nc.tensor.transpose(slabT_ps[:], slab_sb[:], ident[:])
            slabT = rank_pool.tile([BL, B], F32, tag="slabT")
            nc.scalar.copy(slabT[:], slabT_ps[:])
            nc.scalar.dma_start(ag2_in[:], slabT[:])
            if os.environ.get("KTIME"):
                nc.scalar.dma_start(ag2_out[0:BL, :], ag2_in[:])
            else:
                nc.gpsimd.collective_compute(
                    "AllGather",
                    AluOpType.bypass,
                    replica_groups=[list(range(N_CORES))],
                    ins=[ag2_in.opt()],
                    outs=[ag2_out.opt()],
                )

            # gathered rows ARE G^T; G follows by one PE transpose, so all
            # cores share bitwise-identical G and G^T
            Gt_sb = rank_pool.tile([B, B], F32, tag="Gt_sb")
            nc.scalar.dma_start(Gt_sb[:], ag2_out[:])
            G_ps = pst([B, B])
            nc.tensor.transpose(G_ps[:], Gt_sb[:], ident[:])
            G_sb = rank_pool.tile([B, B], F32, tag="G_sb")
            nc.scalar.copy(G_sb[:], G_ps[:])

            if stage == "rank1":
                nc.sync.dma_start(dbg_d[:, 0:B], G_sb[:])
                return

            junk = rank_pool.tile([B, B], F32, tag="junk")
            diag = rank_pool.tile([B, 1], F32, tag="diag")
            nc.vector.tensor_tensor(junk[:], G_sb[:], ident[:], AluOpType.mult)
            nc.vector.reduce_sum(diag[:], junk[:], axis=AX.X)
            mdiag = rank_pool.tile([B, 1], F32, tag="mdiag")
            nc.vector.tensor_scalar(mdiag[:], diag[:], -1.0, MARGIN,
                                    AluOpType.mult, AluOpType.add)

            stack = rank_pool.tile([B, 6], F32, tag="stack")

            for di, Gsrc in enumerate((G_sb, Gt_sb)):
                # two independent per-direction chains on separate engines
                eng = nc.vector if di == 0 else nc.gpsimd
                junk_d = rank_pool.tile([B, B], F32, tag=f"junk{di}", name="junk_d")
                Gm = rank_pool.tile([B, B], F32, tag=f"Gm{di}", name="Gm")
                rmax = rank_pool.tile([B, 1], F32, tag=f"rmax{di}", name="rmax")
                top = rank_pool.tile([B, 1], F32, tag=f"top{di}", name="top")
                w = rank_pool.tile([B, 1], F32, tag=f"w{di}", name="w")
                sel = rank_pool.tile([B, 1], F32, tag=f"sel{di}", name="sel")
                eq = rank_pool.tile([B, 1], F32, tag=f"eq{di}", name="eq")
                colv = rank_pool.tile([B, 1], F32, tag=f"colv{di}", name="colv")
                T_sb = rank_pool.tile([B, B], F32, tag=f"T{di}")
                nc.scalar.activation(T_sb[:], Gsrc[:], AF.Relu, bias=mdiag[:])
                eng.tensor_tensor(junk_d[:], T_sb[:], gb_sb[:], AluOpType.mult)
                nc.vector.reduce_sum(w[:], junk_d[:], axis=AX.X)
                nc.vector.tensor_tensor(stack[:, di : di + 1], w[:], g_col[:], AluOpType.mult)
                eng.tensor_tensor(Gm[:], Gsrc[:], gneg_sb[:], AluOpType.add)
                nc.vector.reduce_max(rmax[:], Gm[:], axis=AX.X)
                nc.vector.tensor_tensor(top[:], diag[:], rmax[:], AluOpType.is_ge)
                eng.tensor_tensor(junk_d[:], Gsrc[:], gb_sb[:], AluOpType.mult)
                nc.vector.reduce_sum(sel[:], junk_d[:], axis=AX.X)
                nc.vector.tensor_tensor(sel[:], sel[:], g_col[:], AluOpType.mult)
                nc.vector.tensor_scalar(eq[:], sel[:], 0.0, None, AluOpType.is_equal)
                nc.vector.tensor_scalar(colv[:], eq[:], -1.0, 1.0,
                                        AluOpType.mult, AluOpType.add)
                nc.vector.tensor_copy(stack[:, 4 + di : 5 + di], colv[:])
                nc.vector.tensor_tensor(stack[:, 2 + di : 3 + di], colv[:], top[:],
                                        AluOpType.mult)

            if stage == "rank2":
                nc.sync.dma_start(dbg_d[:, 0:6], stack[:])
                nc.sync.dma_start(dbg_d[:, 8:136], Gt_sb[:])
                return

            S_ps = pst([1, 6])
            nc.tensor.matmul(S_ps[:], ones_col_f32[:], stack[:], start=True, stop=True)
            S_sb = rank_pool.tile([1, 6], F32, tag="S_sb")
            nc.vector.tensor_copy(S_sb[:], S_ps[:])

            sg = rank_pool.tile([1, 1], F32, tag="sg")
            nc.vector.reduce_sum(sg[:], g_row[:], axis=AX.X)

            def sc(tag):
                return rank_pool.tile([1, 1], F32, tag=tag, name=tag)

            t_ls = sc("t_ls")
            nc.vector.tensor_tensor(t_ls[:], S_sb[:, 0:1], S_sb[:, 1:2], AluOpType.add)
            num = sc("num")
            nc.vector.tensor_scalar_mul(num[:], sg[:], -2.0 * MARGIN)
            nc.vector.tensor_tensor(num[:], num[:], t_ls[:], AluOpType.add)
            d1 = sc("d1")
            nc.vector.tensor_scalar(d1[:], sg[:], -1.0, 1.0, AluOpType.add, AluOpType.max)
            ind = sc("ind")
            nc.vector.tensor_scalar(ind[:], sg[:], -1.0, 0.0, AluOpType.add, AluOpType.max)
            nc.vector.tensor_scalar_min(ind[:], ind[:], 1.0)
            nv = sc("nv")
            nc.vector.tensor_tensor(nv[:], ind[:], sg[:], AluOpType.mult)
            d2 = sc("d2")
            nc.vector.tensor_scalar_max(d2[:], nv[:], 1.0)
            r1 = sc("r1")
            nc.vector.reciprocal(r1[:], d1[:])
            r2 = sc("r2")
            nc.vector.reciprocal(r2[:], d2[:])
            out_sb = rank_pool.tile([1, 2], F32, tag="out_sb")
            nc.vector.tensor_tensor(num[:], num[:], r1[:], AluOpType.mult)
            nc.vector.tensor_tensor(out_sb[:, 0:1], num[:], r2[:], AluOpType.mult)

            acc_h = []
            for di in range(2):
                nvx = sc(f"nvx{di}")
                nc.vector.tensor_scalar_max(nvx[:], S_sb[:, 4 + di : 5 + di], 1.0)
                rx = sc(f"rx{di}")
                nc.vector.reciprocal(rx[:], nvx[:])
                ax = sc(f"ax{di}")
                nc.vector.tensor_tensor(ax[:], S_sb[:, 2 + di : 3 + di], rx[:], AluOpType.mult)
                acc_h.append(ax)
            asum = sc("asum")
            nc.vector.tensor_tensor(asum[:], acc_h[0][:], acc_h[1][:], AluOpType.add)
            nc.vector.tensor_scalar_mul(out_sb[:, 1:2], asum[:], 0.5)

            nc.sync.dma_start(out_d[:], out_sb[:])


def _encoder_gen(nc, tc, pst, const, mod, dm, x_d, wd, wt, feat_sb, feat_off,
                 ident, ones_row_r, ones64_bf, shared):
    """Self-attention pooled encoder as a generator: yields at chunk
    boundaries so the driver can interleave the two modalities. x rides the
    sync HWDGE queue; small loads ride scalar. qkv weights (wt) are bf16
    tiles staged by the driver via SWDGE cast loads.

    Writes feat_sb[:, feat_off:feat_off+512].
    """
    K = dm // 128
    n_tt = TOK // 128  # 8
    wv, wq, wk = wt[f"{mod}_v"], wt[f"{mod}_q"], wt[f"{mod}_k"]

    enc_cm = tc.tile_pool(name=f"enc_{mod}", bufs=1)
    enc = enc_cm.__enter__()

    estk = ExitStack()
    xT_pool = estk.enter_context(tc.tile_pool(name=f"xT_{mod}", bufs=1))
    # rgb runs the bf16 matmul path (pipelined weight loads); audio stays
    # f32r end-to-end (dtypes must match within a matmul)
    xT = xT_pool.tile([128, K, TOK], BF16 if dm != 128 else F32R, tag="xT")
    flat = x_d.rearrange("b t d -> (b t) d")

    bv = const.tile([1, DIM], F32R, tag=f"bv_{mod}")
    nc.scalar.dma_start(bv[:], wd[f"{mod}_bv"][None, :])
    bq_sb = const.tile([128, HEADS], F32R, tag=f"bq_{mod}")
    nc.scalar.dma_start(bq_sb[:], wd[f"{mod}_bq"].rearrange("(o p) -> p o", p=128))
    bk_sb = const.tile([128, HEADS], F32R, tag=f"bk_{mod}")
    nc.scalar.dma_start(bk_sb[:], wd[f"{mod}_bk"].rearrange("(o p) -> p o", p=128))

    xload_cm = tc.tile_pool(name=f"xload_{mod}", bufs=2)
    xload = xload_cm.__enter__()
    if dm == 128:
        x_nat_a = xload.tile([128, n_tt, 128], F32, tag="x_nat_a")
        nc.sync.dma_start(x_nat_a[:], flat.rearrange("(n p) d -> p n d", p=128))
        for tt in range(n_tt):
            tp = pst([128, 512])
            nc.tensor.transpose(tp[:, :128], x_nat_a[:, tt, :], ident[:])
            nc.scalar.copy(xT[:, 0, 128 * tt : 128 * (tt + 1)], tp[:, :128])
            if tt % 4 == 3:
                yield "xT"
    else:
        x_nats = [xload.tile([128, dm], F32, tag="x_nat", name="x_nat")]
        nc.sync.dma_start(x_nats[0][:], flat[0:128, :])
        for tt in range(n_tt):
            if tt + 1 < n_tt:
                nxt = xload.tile([128, dm], F32, tag="x_nat", name="x_nat")
                nc.sync.dma_start(nxt[:], flat[128 * (tt + 1) : 128 * (tt + 2), :])
                x_nats.append(nxt)
            x_nat = x_nats[tt]
            for kc4 in range(K // 4):
                tp = pst([128, 512])
                for j in range(4):
                    kc = 4 * kc4 + j
                    nc.tensor.transpose(tp[:, 128 * j : 128 * (j + 1)],
                                        x_nat[:, 128 * kc : 128 * (kc + 1)], ident[:])
                nc.scalar.copy(xT[:, 4 * kc4 : 4 * kc4 + 4, 128 * tt : 128 * (tt + 1)],
                               tp[:].rearrange("p (j c) -> p j c", j=4))
                yield "xT"
    xload_cm.__exit__(None, None, None)

    # v: lhsT = xT token-tile (stationary), rhs = Wv k-rows (moving), all bf16
    v_sb = enc.tile([128, n_tt, DIM], BF16, tag="v_sb")
    for tt in range(n_tt):
        pv = pst([128, DIM])
        for kc in range(K):
            nc.tensor.matmul(pv[:], xT[:, kc, 128 * tt : 128 * (tt + 1)], wv[:, kc, :],
                             start=(kc == 0), stop=False)
        nc.tensor.matmul(pv[:], ones_row_r[:], bv[:], start=False, stop=True)
        nc.vector.tensor_copy(v_sb[:, tt, :], pv[:])
        yield "v"

    # q, k: lhsT = W column-block (stationary), rhs = xT (moving) -> [d, tok]
    qT = enc.tile([128, HEADS, TOK], BF16, tag="qT")
    kT = enc.tile([128, HEADS, TOK], BF16, tag="kT")
    for wfull, outT, b_sb in ((wq, qT, bq_sb), (wk, kT, bk_sb)):
        for dt_ in range(HEADS):
            for blk in range(TOK // 512):
                pq = pst([128, 512])
                for kc in range(K):
                    nc.tensor.matmul(pq[:], wfull[:, kc, 128 * dt_ : 128 * (dt_ + 1)],
                                     xT[:, kc, 512 * blk : 512 * (blk + 1)],
                                     start=(kc == 0), stop=(kc == K - 1))
                nc.scalar.activation(outT[:, dt_, 512 * blk : 512 * (blk + 1)], pq[:],
                                     AF.Identity, bias=b_sb[:, dt_ : dt_ + 1])
                yield "qk"
    estk.close()  # frees xT
    yield "attn_start"

    # attention, grp-outer: reshuffle 8 samples of v to base partition 0 via
    # SBUF->SBUF DMA (engines cannot shift partitions), then per-head flow.
    # Time-pooling happens here: av psum is DVE-reduced straight into pooled.
    scale = 1.0 / math.sqrt(HD)
    # pooled attention output (time-summed), per head: [hd, h, samples]
    pooled = enc.tile([128, HEADS, BL], F32R, tag="pooled")
    lstk = ExitStack()
    wo_pool = lstk.enter_context(tc.tile_pool(name=f"wo_{mod}", bufs=1))
    wo = wo_pool.tile([128, HEADS, DIM], F32R, tag="wo")
    nc.scalar.dma_start(wo[:], wd[f"{mod}_Wo"].rearrange("(h p) d -> p h d", p=128))
    w2 = wo_pool.tile([128, HEADS, DIM], F32R, tag="w2")
    nc.scalar.dma_start(w2[:], wd[f"{mod}_W2"].rearrange("(c p) d -> p c d", p=128))
    b2 = wo_pool.tile([1, DIM], F32R, tag="b2")
    nc.scalar.dma_start(b2[:], wd[f"{mod}_b2"][None, :])
    boT = wo_pool.tile([128, HEADS], F32R, tag="boT")
    nc.scalar.dma_start(boT[:], wd[f"{mod}_bo"].rearrange("(o p) -> p o", p=128))
    # b2' = bo @ W2 + b2 (folds the out-proj bias through W2)
    b2p_ps = pst([1, DIM])
    for c in range(HEADS):
        nc.tensor.matmul(b2p_ps[:], boT[:, c : c + 1], w2[:, c, :],
                         start=(c == 0), stop=False)
    nc.tensor.matmul(b2p_ps[:], ones_row_r[:, 0:1], b2[:], start=False, stop=True)
    b2p = wo_pool.tile([1, DIM], F32R, tag="b2p")
    nc.scalar.copy(b2p[:], b2p_ps[:])
    pe_ns(300)
    yield "w"

    ap = shared["attn_pool"]  # driver-owned shared scratch
    for grp in range(BL // 8):
        v8 = ap.tile([64, 8, DIM], BF16, tag="v8", bufs=2)
        v8v = v8[:].rearrange("p (ul half) d -> p ul half d", half=2)
        nc.scalar.dma_start(v8v[:, :, 0, :], v_sb[0:64, 4 * grp : 4 * grp + 4, :])
        nc.scalar.dma_start(v8v[:, :, 1, :], v_sb[64:128, 4 * grp : 4 * grp + 4, :])
        for h in range(HEADS):
            sT8 = pst([64, 512], tag="ps_s", bufs=2)
            for i in range(8):
                b = 8 * grp + i
                nc.tensor.matmul(sT8[:, 64 * i : 64 * (i + 1)],
                                 kT[:, h, 64 * b : 64 * (b + 1)],
                                 qT[:, h, 64 * b : 64 * (b + 1)],
                                 start=True, stop=True)
            exps = ap.tile([64, 512], BF16, tag="exps")
            nc.scalar.activation(exps[:], sT8[:], AF.Exp, scale=scale)
            pe_ns(500)
            yield "attn_s"
            rs = pst([64, 512], tag="ps_rs", bufs=1)
            nc.tensor.matmul(rs[:], ones64_bf[:], exps[:], start=True, stop=True)
            rrs = ap.tile([64, 512], F32, tag="rrs")
            nc.vector.reciprocal_approx_fast(rrs[:], rs[:])
            aT8 = ap.tile([64, 512], BF16, tag="aT8")
            nc.vector.tensor_tensor(aT8[:], exps[:], rrs[:], AluOpType.mult)
            pe_ns(250)
            yield "attn_m"
            avp = pst([128, 512])
            for i in range(8):
                nc.tensor.matmul(avp[:, 64 * i : 64 * (i + 1)],
                                 v8[:, i, 128 * h : 128 * (h + 1)],
                                 aT8[:, 64 * i : 64 * (i + 1)],
                                 start=True, stop=True)
            # time-pool the attention output for these 8 samples directly
            # from psum: [128, (8s 64t)] -> [128, 8]
            with nc.allow_low_precision(reason="f32r is bit-identical to f32; accumulation is full fp32"):
                nc.vector.reduce_sum(pooled[:, h, 8 * grp : 8 * grp + 8],
                                     avp[:].rearrange("p (s t) -> p s t", t=T),
                                     axis=AX.X)
            pe_ns(600)
            yield "attn_v"

    # out-proj on pooled sums: out1 = (pooled/T) @ Wo  (bo folded into b2')
    out1_ps = pst([BL, DIM])
    for h in range(HEADS):
        nc.tensor.matmul(out1_ps[:], pooled[:, h, :], wo[:, h, :],
                         start=(h == 0), stop=(h == HEADS - 1))
    out1 = shared["out1_scr"]
    nc.scalar.activation(out1[:], out1_ps[:], AF.Copy, scale=1.0 / T)
    pe_ns(900)
    yield "proj"

    out1T = wo_pool.tile([128, HEADS, BL], F32R, tag="out1T")
    for c in range(HEADS):
        tp = pst([128, BL])
        nc.tensor.transpose(tp[:], out1[:, 128 * c : 128 * (c + 1)], ident[:BL, :BL])
        nc.scalar.copy(out1T[:, c, :], tp[:])
    pf = pst([BL, DIM])
    for c in range(HEADS):
        nc.tensor.matmul(pf[:], out1T[:, c, :], w2[:, c, :], start=(c == 0), stop=False)
    nc.tensor.matmul(pf[:], ones_row_r[:, :BL], b2p[:], start=False, stop=True)
    nc.scalar.copy(feat_sb[:, feat_off : feat_off + DIM], pf[:])
    pe_ns(1500)
    yield "end"
    lstk.close()
    enc_cm.__exit__(None, None, None)


def kernel(**inputs):
    if "runner" not in _CACHE:
        _CACHE["runner"] = _make_runner()
    return _CACHE["runner"](inputs)


def _make_runner():
    nc = _build()
    import jax
    from jax.sharding import Mesh, PartitionSpec
    from jax.experimental.shard_map import shard_map
    from concourse import bass2jax

    bass2jax.install_neuronx_cc_hook()

    partition_name = nc.partition_id_tensor.name if nc.partition_id_tensor else None
    in_names, out_names, out_avals, zero_outs = [], [], [], []
    for alloc in nc.m.functions[0].allocations:
        if not isinstance(alloc, mybir.MemoryLocationSet):
            continue
        name = alloc.memorylocations[0].name
        if alloc.kind == "ExternalInput":
            if name != partition_name:
                in_names.append(name)
        elif alloc.kind == "ExternalOutput":
            out_names.append(name)
            shape = tuple(alloc.tensor_shape)
            dtype = mybir.dt.np(alloc.dtype)
            out_avals.append(jax.core.ShapedArray(shape, dtype))
            zero_outs.append(np.zeros(shape, dtype))
    n_params = len(in_names)
    all_in_names = list(in_names) + list(out_names)
    if partition_name is not None:
        all_in_names.append(partition_name)

    def _body(*args):
        operands = list(args)
        if partition_name is not None:
            operands.append(bass2jax.partition_id_tensor())
        outs = bass2jax._bass_exec_p.bind(
            *operands,
            out_avals=tuple(out_avals),
            in_names=tuple(all_in_names),
            out_names=tuple(out_names),
            lowering_input_output_aliases=(),
            sim_require_finite=True,
            sim_require_nnan=True,
            nc=nc,
        )
        return tuple(outs)

    devices = jax.devices()[:N_CORES]
    mesh = Mesh(np.asarray(devices), ("core",))
    in_specs = (PartitionSpec("core"),) * (n_params + len(out_names))
    out_specs = (PartitionSpec("core"),) * len(out_names)
    sharded = jax.jit(
        shard_map(_body, mesh=mesh, in_specs=in_specs, out_specs=out_specs,
                  check_rep=False),
        keep_unused=True,
    )

    out_idx = out_names.index("out")

    def run(inputs):
        per_core = _shard_inputs(inputs)
        concat_in = [
            np.concatenate([per_core[c][name] for c in range(N_CORES)], axis=0)
            for name in in_names
        ]
        concat_zeros = [
            np.zeros((N_CORES * z.shape[0], *z.shape[1:]), z.dtype) for z in zero_outs
        ]
        out_arrs = sharded(*concat_in, *concat_zeros)
        run.last_outputs = {n: np.asarray(out_arrs[i]) for i, n in enumerate(out_names)}
        out = run.last_outputs["out"]  # [8, 2]
        return np.float32(out[0, 0]), np.float32(out[0, 1])

    run.sharded = sharded
    run.in_names = in_names
    run.zero_outs = zero_outs
    run.nc = nc
    return run


def _shard_inputs(inputs):
    per_core = []
    gm = np.ascontiguousarray(np.asarray(inputs["group_mask"]).astype(np.uint8))
    shared = {}
    for k, v in inputs.items():
        if k not in ("o", "rgb", "audio", "group_mask"):
            shared[k] = np.ascontiguousarray(np.asarray(v, dtype=np.float32))
    o = np.asarray(inputs["o"], dtype=np.float32)
    rgb = np.asarray(inputs["rgb"], dtype=np.float32)
    audio = np.asarray(inputs["audio"], dtype=np.float32)
    for c in range(N_CORES):
        sl = slice(BL * c, BL * (c + 1))
        m = {
            "o": np.ascontiguousarray(o[sl]),
            "rgb": np.ascontiguousarray(rgb[sl]),
            "audio": np.ascontiguousarray(audio[sl]),
            "group_mask": gm,
        }
        m.update(shared)
        per_core.append(m)
    return per_core
